# revision 13
# baseline (speedup 1.0000x reference)
"""Trainium2 Bass kernel for 4-layer bidirectional GRU (H=128, T=200) + MLP head.

Data-parallel over the 400 flattened sequences -> 50 per core on 8 cores.
Layout: 128 partitions = hidden unit, free dim = batch slots.

v2: dual-stream latency hiding (fwd and bwd run as two INDEPENDENT chains
of width 50, phase-shifted by half a step) instead of one fused width-100
chain.  The per-stream serial chain (sig -> r*Q -> ident-matmul -> tanh ->
a-mult -> a-matmuls) is ~1.45us; the other stream's ops ride in its engine
gaps, so one full timestep (both directions) completes per ~1.5us instead
of ~2.24us for the fused chain.

  - PSUM: per stream per chunk two single-bank tiles, slots interleaved
    per step: RZ bank [r_t | z_t]*ct, NQ bank [n_t | q_t]*ct (ct=5, 100
    f32 per step, 500 <= 512).  2 tiles x 2 streams x 2 bufs = 8 banks.
    One start (first gi matmul) / one stop (last writer) per bank per
    chunk; first touch of a pending byte overwrites, later ones accumulate
    (zero-region = whole bank), so q slots need no prefill at all.
  - Bias folding: tanh bias (bih_n) rides the ACT bias operand
    (per-partition column); q bias (bhh_n) rides the scalar of a
    scalar_tensor_tensor: tmp = (Q + bhh_n) * r.  Only [r|z] biases need a
    prefill matmul (one K=2 mask matmul per chunk).  z is negated
    end-to-end so ONE sigmoid covers [r|zbar].
  - h' = a + b with a = zbar*n, b = z*h = h - zbar*h: the recurrent
    Whh@h' is accumulated as separate matmuls on a and b, so only the
    3 a-matmuls sit on the chain after tanh; the b-matmuls (ready right
    after the sigmoid) hide inside the tanh window.
  - u = zbar*h and b = h - u run on the (otherwise idle) GPSIMD engine.
  - Emission order interleaves the streams so the ACT FIFO per round is
    [F.sig(t), B.tanh(t-1), F.tanh(t), B.sig(t)] - B's chain sits exactly
    in F's dependency gaps.
  - Layer 3 runs forward-only plus a single backward step (h0=0 gives the
    last-timestep backward output directly).
"""

import os
import sys

import numpy as np

_REPO = "/opt/trn_rl_repo"
if _REPO not in sys.path:
    sys.path.insert(0, _REPO)

B, KSEQ, T = 4, 100, 200
H = 128
L = 4
OUT = 8
NCORES = 8
N = B * KSEQ              # 400 sequences
NB = N // NCORES          # 50 per core
CT = 5                    # timesteps per PSUM chunk (CT*2*NB = 500 <= 512)
F16 = "float16"

_CACHE = {}


def _build_program(t_len=T, nb=NB, ct=CT):
    import concourse.bacc as bacc
    import concourse.mybir as mybir
    import concourse.tile as tile
    from contextlib import ExitStack

    f32 = mybir.dt.float32
    f16 = mybir.dt.float16

    nch = t_len // ct           # 40 chunks
    W = 2 * nb                  # 100 slot width per step in a PSUM bank

    nc = bacc.Bacc("TRN2", target_bir_lowering=False, debug=False,
                   num_devices=NCORES)

    # ---- DRAM I/O ----
    dx0f = nc.dram_tensor("x0f", (2, t_len * nb), f16, kind="ExternalInput").ap()
    dx0r = nc.dram_tensor("x0r", (2, t_len * nb), f16, kind="ExternalInput").ap()
    dw0 = nc.dram_tensor("w0", (2, 6 * H), f16, kind="ExternalInput").ap()
    dwih = nc.dram_tensor("wihT", (36, H, H), f16, kind="ExternalInput").ap()
    dwhh = nc.dram_tensor("whhT", (24, H, H), f16, kind="ExternalInput").ap()
    dbrz = nc.dram_tensor("brz", (3, 16 * H), f16, kind="ExternalInput").ap()
    dmask = nc.dram_tensor("mask", (2, ct * W), f16, kind="ExternalInput").ap()
    dbihn = nc.dram_tensor("bihn", (H, 8), f32, kind="ExternalInput").ap()
    dident = nc.dram_tensor("ident", (H, H), f16, kind="ExternalInput").ap()
    dw1 = nc.dram_tensor("w1T", (2, H, H), f16, kind="ExternalInput").ap()
    db1 = nc.dram_tensor("b1col", (H, 1), f32, kind="ExternalInput").ap()
    dw2 = nc.dram_tensor("w2T", (H, OUT), f32, kind="ExternalInput").ap()
    db2 = nc.dram_tensor("b2col", (OUT, 1), f32, kind="ExternalInput").ap()
    dout = nc.dram_tensor("out", (OUT, nb), f32, kind="ExternalOutput").ap()

    with tile.TileContext(nc) as tc, ExitStack() as ctx:
        cpool = ctx.enter_context(tc.tile_pool(name="consts", bufs=1))
        xpool = ctx.enter_context(tc.tile_pool(name="xcat", bufs=1))
        spool = ctx.enter_context(tc.tile_pool(name="scratch", bufs=3))

        # ---- constants / weights to SBUF ----
        w0_sb = cpool.tile([2, 6 * H], f16)
        nc.sync.dma_start(w0_sb[:], dw0)
        wih_sb = cpool.tile([H, 36 * H], f16)
        nc.sync.dma_start(wih_sb[:].rearrange("p (i c) -> p i c", c=H),
                          dwih.rearrange("i p c -> p i c"))
        whh_sb = cpool.tile([H, 24 * H], f16)
        nc.sync.dma_start(whh_sb[:].rearrange("p (i c) -> p i c", c=H),
                          dwhh.rearrange("i p c -> p i c"))
        brz_sb = cpool.tile([2, 16 * H], f16)
        nc.sync.dma_start(brz_sb[:], dbrz[0:2])
        bq_sb = cpool.tile([1, 16 * H], f16)
        nc.sync.dma_start(bq_sb[:], dbrz[2:3])
        mask_sb = cpool.tile([2, ct * W], f16)
        nc.sync.dma_start(mask_sb[:], dmask)
        bihn_sb = cpool.tile([H, 8], f32)
        nc.sync.dma_start(bihn_sb[:], dbihn)
        id_sb = cpool.tile([H, H], f16)
        nc.sync.dma_start(id_sb[:], dident)
        w1_sb = cpool.tile([H, 2 * H], f16)
        nc.sync.dma_start(w1_sb[:].rearrange("p (i c) -> p i c", c=H),
                          dw1.rearrange("i p c -> p i c"))
        b1_sb = cpool.tile([H, 1], f32)
        nc.sync.dma_start(b1_sb[:], db1)
        w2_sb = cpool.tile([H, OUT], f32)
        nc.sync.dma_start(w2_sb[:], dw2)
        b2_sb = cpool.tile([OUT, 1], f32)
        nc.sync.dma_start(b2_sb[:], db2)
        h0_sb = cpool.tile([H, nb], f16)
        nc.vector.memset(h0_sb[:], 0.0)
        xA = xpool.tile([H, t_len * W], f16, tag="xA")
        xB = xpool.tile([H, t_len * W], f16, tag="xB")

        def wih_t(l, d, g, k):  # layers 1..3
            i = (((l - 1) * 2 + d) * 3 + g) * 2 + k
            return wih_sb[:, i * H:(i + 1) * H]

        def whh_t(l, d, g):
            i = (l * 2 + d) * 3 + g
            return whh_sb[:, i * H:(i + 1) * H]

        def brz_t(l, d):          # rows 0/1 = [r|z] biases (layers 1-3)
            i = l * 2 + d
            return brz_sb[:, i * H:(i + 1) * H]

        def bq_t(l, d):           # q bias row (all layers)
            i = l * 2 + d
            return bq_sb[:, i * H:(i + 1) * H]

        def col(tile_sb, l, d):
            i = l * 2 + d
            return tile_sb[:, i:i + 1]

        # ------------------------------------------------------------------
        class Stream:
            """One GRU scan direction on one layer."""

            def __init__(self, l, d, x_in, x_out, x0_sb, steps):
                self.l, self.d = l, d
                self.x_in, self.x_out = x_in, x_out
                self.x0_sb = x0_sb
                self.steps = steps
                self.tiles = {}       # c -> (rz_tile, rz_off, nq_tile, nq_off)
                self.pending = []
                self.h = h0_sb[:]
                self.a = None
                self.b = h0_sb[:]
                self.n_sb = None
                self.rz_sb = None

            def prefill(self, c, rz, ro, nq, no):
                """Record chunk c's PSUM regions + build prefill thunks."""
                if c * ct >= self.steps:
                    return []
                l, d = self.l, self.d
                ctc = min(ct, self.steps - c * ct)
                self.tiles[c] = (rz, ro, nq, no)
                thunks = []
                rz3 = rz[:, ro:ro + ct * W].rearrange(
                    "p (t x) -> p t x", x=W)[:, 0:ctc]
                nq3 = nq[:, no:no + ct * W].rearrange(
                    "p (t x) -> p t x", x=W)[:, 0:ctc]
                # q-bias: K=1 matmul over the q slots (ones ride the
                # r-indicator mask positions)
                qmask = mask_sb[0:1, 0:ctc * W].rearrange(
                    "k (t x) -> k t x", x=W)[:, :, 0:nb]
                qbias = (lambda out=nq3[:, :, nb:W], lhsT=bq_t(l, d),
                         rhs=qmask: nc.tensor.matmul(out, lhsT, rhs,
                                                     start=False, stop=False))
                if l == 0:
                    # layer 0: K=2 matmuls carry weights+biases (w0 rows)
                    for g, out in ((0, rz3[:, :, 0:nb]), (1, rz3[:, :, nb:W]),
                                   (2, nq3[:, :, 0:nb])):
                        rhs = self.x0_sb[:, c * ct * nb:(c * ct + ctc) * nb] \
                            .rearrange("p (t n) -> p t n", n=nb)
                        lhsT = w0_sb[:, (d * 3 + g) * H:(d * 3 + g + 1) * H]
                        st = g in (0, 2)    # one start per bank
                        thunks.append(lambda out=out, lhsT=lhsT, rhs=rhs, st=st:
                                      nc.tensor.matmul(out, lhsT, rhs,
                                                       start=st, stop=False))
                    thunks.append(qbias)
                else:
                    s0 = c * ct
                    hi = t_len - 1 - s0
                    lo = hi - ctc
                    asc = slice(s0, s0 + ctc)
                    dsc = slice(hi, lo if lo >= 0 else None, -1)
                    x3 = self.x_in[:].rearrange("p (t w) -> p t w", w=W)
                    for g, out in ((0, rz3[:, :, 0:nb]), (1, rz3[:, :, nb:W]),
                                   (2, nq3[:, :, 0:nb])):
                        for k in (0, 1):
                            sl = asc if k == d else dsc
                            rr = x3[:, sl, k * nb:(k + 1) * nb]
                            st = (g in (0, 2)) and k == 0
                            thunks.append(
                                lambda out=out, lhsT=wih_t(l, d, g, k), rr=rr,
                                st=st: nc.tensor.matmul(out, lhsT, rr,
                                                        start=st, stop=False))
                    # [r|z] biases: one K=2 mask matmul for the whole chunk
                    bw = ctc * W
                    thunks.append(lambda rz=rz, ro=ro, lhsT=brz_t(l, d),
                                  bw=bw:
                                  nc.tensor.matmul(rz[:, ro:ro + bw], lhsT,
                                                   mask_sb[:, 0:bw],
                                                   start=False, stop=False))
                    thunks.append(qbias)
                return thunks

            def pop(self, n):
                for _ in range(n):
                    if self.pending:
                        self.pending.pop(0)()

            # --- step phases (s = scan position) ---
            def mm_b(self, s):
                """Recurrent matmuls on b (and h0 at s=0) into slot s."""
                l, d = self.l, self.d
                c, tl = divmod(s, ct)
                rz, ro, nq, no = self.tiles[c]
                single = (self.steps == 1)
                for g, bank, off in ((0, rz, ro), (2, nq, no + nb),
                                     (1, rz, ro + nb)):
                    out = bank[:, tl * W + off:tl * W + off + nb]
                    stop = single and g == 1
                    nc.tensor.matmul(out, whh_t(l, d, g), self.b,
                                     start=False, stop=stop)

            def mm_a(self, s):
                if self.a is None:
                    return
                l, d = self.l, self.d
                c, tl = divmod(s, ct)
                rz, ro, nq, no = self.tiles[c]
                last = (tl == ct - 1) or (s == self.steps - 1)
                for g, bank, off in ((2, nq, no + nb), (0, rz, ro),
                                     (1, rz, ro + nb)):
                    out = bank[:, tl * W + off:tl * W + off + nb]
                    stop = last and g == 1   # z a-matmul closes its bank
                    nc.tensor.matmul(out, whh_t(l, d, g), self.a,
                                     start=False, stop=stop)

            def sig(self, s):
                c, tl = divmod(s, ct)
                rz, ro, _, _ = self.tiles[c]
                if ro == 0:
                    self.rz_sb = spool.tile([H, W], f16, tag=f"rz{self.d}")
                    self.rz0 = 0
                    nc.scalar.activation(self.rz_sb[:],
                                         rz[:, tl * W:tl * W + W],
                                         mybir.ActivationFunctionType.Sigmoid)
                else:
                    # read [F's nq slot | own rz slot] from the shared 2-bank
                    # tile: phase lock - this sig can't start before the
                    # other stream's ident-matmul of the same round is done
                    self.rz_sb = spool.tile([H, 2 * W], f16,
                                            tag=f"rz{self.d}")
                    self.rz0 = W
                    src_ap = rz[:, 0:1024].rearrange(
                        "p (b x) -> p b x", b=2)[:, :, tl * W:tl * W + W]
                    nc.scalar.activation(
                        self.rz_sb[:].rearrange("p (b x) -> p b x", b=2),
                        src_ap, mybir.ActivationFunctionType.Sigmoid)

            def tmp(self, s):
                """tmp = Q * r on DVE, then an ident-matmul accumulates it
                onto the N slot (PSUM f32) - chain hops DVE -> PE -> ACT so
                the two in-phase streams pipeline one engine apart."""
                c, tl = divmod(s, ct)
                _, _, nq, no = self.tiles[c]
                last = (tl == ct - 1) or (s == self.steps - 1)
                t_sb = spool.tile([H, nb], f16, tag=f"tmp{self.d}")
                nc.vector.tensor_tensor(
                    t_sb[:], nq[:, no + tl * W + nb:no + tl * W + W],
                    self.rz_sb[:, self.rz0:self.rz0 + nb],
                    op=mybir.AluOpType.mult)
                nc.tensor.matmul(nq[:, no + tl * W:no + tl * W + nb], id_sb[:],
                                 t_sb[:], start=False, stop=last)

            def tanh(self, s):
                c, tl = divmod(s, ct)
                _, _, nq, no = self.tiles[c]
                self.n_sb = spool.tile([H, nb], f16, tag=f"n{self.d}")
                nc.scalar.activation(self.n_sb[:],
                                     nq[:, no + tl * W:no + tl * W + nb],
                                     mybir.ActivationFunctionType.Tanh,
                                     bias=col(bihn_sb, self.l, self.d))
                if tl == ct - 1 or s == self.steps - 1:
                    del self.tiles[c]

            def amul(self, s):
                a_sb = spool.tile([H, nb], f16, tag=f"a{self.d}")
                zb = self.rz_sb[:, self.rz0 + nb:self.rz0 + W]
                nc.vector.tensor_tensor(a_sb[:], zb, self.n_sb[:],
                                        op=mybir.AluOpType.mult)
                self.a = a_sb[:]

            def ub(self, s):
                """u = zbar*h, b = h - u on GPSIMD (off the critical chain)."""
                u_sb = spool.tile([H, nb], f16, tag=f"u{self.d}")
                zb = self.rz_sb[:, self.rz0 + nb:self.rz0 + W]
                nc.gpsimd.tensor_tensor(u_sb[:], zb, self.h,
                                        op=mybir.AluOpType.mult)
                b_sb = spool.tile([H, nb], f16, tag=f"b{self.d}")
                nc.gpsimd.tensor_tensor(b_sb[:], self.h, u_sb[:],
                                        op=mybir.AluOpType.subtract)
                self.b = b_sb[:]

            def hadd(self, s):
                x3 = self.x_out[:].rearrange("p (t w) -> p t w", w=W)
                h_new = x3[:, s, self.d * nb:(self.d + 1) * nb]
                nc.vector.tensor_tensor(h_new, self.a, self.b,
                                        op=mybir.AluOpType.add)
                self.h = h_new

        # ------------------------------------------------------------------
        def run_layer(l, x_in, x_out, pscan, x0f_sb=None, x0r_sb=None,
                      fwd_only=False):
            F = Stream(l, 0, x_in, x_out, x0f_sb, t_len)
            Bk = Stream(l, 1, x_in, x_out, x0r_sb,
                        1 if fwd_only else t_len)

            def chunk_prefill(c):
                """Allocate chunk c's PSUM: F-rz bank, shared 2-bank tile
                [F.nq | B.rz] (the phase lock), B-nq bank."""
                frz = pscan.tile([H, 512], f32, tag="frz")
                mid = pscan.tile([H, 1024], f32, tag="mid")
                bnq = pscan.tile([H, 512], f32, tag="bnq")
                return (F.prefill(c, frz, 0, mid, 0),
                        Bk.prefill(c, mid, 512, bnq, 0))

            pf, pb = chunk_prefill(0)
            for th in pf:
                th()
            for th in pb:
                th()
            F.pending, Bk.pending = chunk_prefill(1)
            if fwd_only:
                Bk.pending = []

            # bootstrap step 0 (h0 = a0-group = 0 -> b-group only, b(0)=0)
            F.mm_b(0)
            F.pop(2)
            F.sig(0)
            F.tmp(0)
            F.tanh(0)
            F.amul(0)
            F.hadd(0)
            F.b = h0_sb[:]
            Bk.mm_b(0)
            Bk.pop(2)
            Bk.sig(0)
            Bk.tmp(0)
            Bk.b = h0_sb[:]
            if fwd_only:
                # B is a single step: finish it inline
                Bk.tanh(0)
                Bk.amul(0)
                Bk.hadd(0)

            for s in range(1, t_len):
                # refill pending prefill thunks at chunk starts
                tl = s % ct
                if tl == 0:
                    F.pending, Bk.pending = chunk_prefill(s // ct + 1)
                    if fwd_only:
                        Bk.pending = []
                npop = (len(F.pending) + (ct - 1 - tl)) // (ct - tl) \
                    if tl else 2

                # ---- round s: B runs half a step behind F ----
                F.mm_b(s)
                F.mm_a(s)
                F.pop(npop)
                F.sig(s)
                if not fwd_only:
                    Bk.tanh(s - 1)
                    Bk.amul(s - 1)
                F.tmp(s)
                F.tanh(s)
                F.amul(s)
                if not fwd_only:
                    Bk.hadd(s - 1)
                F.ub(s)
                F.hadd(s)
                if not fwd_only:
                    Bk.mm_b(s)
                    Bk.mm_a(s)
                    Bk.pop(npop)
                    Bk.sig(s)
                    Bk.tmp(s)
                    Bk.ub(s)

            if not fwd_only:
                Bk.tanh(t_len - 1)
                Bk.amul(t_len - 1)
                Bk.hadd(t_len - 1)

        # ---------------- layers ----------------
        with tc.tile_pool(name="l0feed", bufs=1) as fpool, \
             tc.tile_pool(name="pscan", bufs=2, space="PSUM") as pscan:
            x0f_sb = fpool.tile([2, t_len * nb], f16)
            nc.sync.dma_start(x0f_sb[:], dx0f)
            x0r_sb = fpool.tile([2, t_len * nb], f16)
            nc.sync.dma_start(x0r_sb[:], dx0r)

            run_layer(0, None, xA, pscan, x0f_sb, x0r_sb)
            run_layer(1, xA, xB, pscan)
            run_layer(2, xB, xA, pscan)
            run_layer(3, xA, xB, pscan, fwd_only=True)

        # ---------------- MLP head ----------------
        xB3 = xB[:].rearrange("p (t w) -> p t w", w=W)
        hf = xB3[:, t_len - 1, 0:nb]
        hb = xB3[:, 0, nb:W]
        with tc.tile_pool(name="phead", bufs=1, space="PSUM") as php:
            ph1 = php.tile([H, nb], f32)
            nc.tensor.matmul(ph1[:], w1_sb[:, 0:H], hf,
                             start=True, stop=False)
            nc.tensor.matmul(ph1[:], w1_sb[:, H:2 * H], hb,
                             start=False, stop=True)
            h1p = spool.tile([H, nb], f32, tag="h1p")
            nc.scalar.activation(h1p[:], ph1[:],
                                 mybir.ActivationFunctionType.Identity,
                                 bias=b1_sb[:])
            h1 = spool.tile([H, nb], f32, tag="h1")
            nc.vector.scalar_tensor_tensor(
                h1[:], h1p[:], 0.2, h1p[:],
                op0=mybir.AluOpType.mult, op1=mybir.AluOpType.max)
            po = php.tile([OUT, nb], f32)
            nc.tensor.matmul(po[:], w2_sb[:], h1[:], start=True, stop=True)
            o_sb = spool.tile([OUT, nb], f32, tag="o_sb")
            nc.scalar.activation(o_sb[:], po[:],
                                 mybir.ActivationFunctionType.Identity,
                                 bias=b2_sb[:])
            nc.sync.dma_start(dout, o_sb[:])

    nc.compile()
    return nc


def _prep_host(raw, Wih0, Wih, Whh, bih, bhh, W1, b1, W2, b2,
               t_len=T, nb=NB, ct=CT):
    """Host-side weight/layout prep. Returns (shared_inputs, per_core_feeds)."""
    f16 = np.float16
    Wih0 = np.asarray(Wih0, np.float32)
    Wih = np.asarray(Wih, np.float32)
    Whh = np.asarray(Whh, np.float32)
    bih = np.asarray(bih, np.float32)
    bhh = np.asarray(bhh, np.float32)

    # The z gate is computed as zbar = sigmoid(-z_preact): negate every
    # z-path weight/bias so the z slots hold -z_preact.
    def zsign(g):
        return -1.0 if g == 1 else 1.0

    # layer0 lhsT (2, 6*128): row0 weights, row1 combined bias
    # (n-gate bias rides the tanh bias operand instead -> row1 = 0)
    w0 = np.zeros((2, 6 * H), np.float32)
    for d in range(2):
        for g in range(3):
            sl = slice(g * H, (g + 1) * H)
            w0[0, (d * 3 + g) * H:(d * 3 + g + 1) * H] = \
                zsign(g) * Wih0[d, sl, 0]
            if g < 2:
                bb = bih[0, d, sl] + bhh[0, d, sl]
                w0[1, (d * 3 + g) * H:(d * 3 + g + 1) * H] = zsign(g) * bb

    wihT = np.zeros((36, H, H), np.float32)
    for l in range(1, 4):
        for d in range(2):
            for g in range(3):
                for k in range(2):
                    i = (((l - 1) * 2 + d) * 3 + g) * 2 + k
                    wihT[i] = zsign(g) * Wih[l - 1, d, g * H:(g + 1) * H,
                                             k * H:(k + 1) * H].T
    whhT = np.zeros((24, H, H), np.float32)
    for l in range(4):
        for d in range(2):
            for g in range(3):
                whhT[(l * 2 + d) * 3 + g] = \
                    zsign(g) * Whh[l, d, g * H:(g + 1) * H, :].T

    # bias lhsT rows per (layer, dir): row0 = r bias, row1 = negated z
    # bias (layers 1-3 only; layer 0 rides w0), row2 = q bias (bhh_n)
    brz = np.zeros((3, 16 * H), np.float32)
    for l in range(4):
        for d in range(2):
            i = l * 2 + d
            if l > 0:
                brz[0, i * H:(i + 1) * H] = bih[l, d, 0:H] + bhh[l, d, 0:H]
                brz[1, i * H:(i + 1) * H] = -(bih[l, d, H:2 * H] +
                                              bhh[l, d, H:2 * H])
            brz[2, i * H:(i + 1) * H] = bhh[l, d, 2 * H:3 * H]

    # mask (2, ct*100): row0 selects r slots (col%100 < 50), row1 z slots
    mask = np.zeros((2, ct * 2 * nb), np.float32)
    m3 = mask.reshape(2, ct, 2 * nb)
    m3[0, :, 0:nb] = 1.0
    m3[1, :, nb:2 * nb] = 1.0

    # per-(layer, dir) tanh bias columns (bih_n)
    bihn = np.zeros((H, 8), np.float32)
    for l in range(4):
        for d in range(2):
            bihn[:, l * 2 + d] = bih[l, d, 2 * H:3 * H]

    shared = {
        "w0": w0.astype(f16),
        "wihT": wihT.astype(f16),
        "whhT": whhT.astype(f16),
        "brz": brz.astype(f16),
        "mask": mask.astype(f16),
        "bihn": bihn,
        "ident": np.eye(H, dtype=f16),
        "w1T": np.stack([np.asarray(W1, np.float32)[:, 0:H].T,
                         np.asarray(W1, np.float32)[:, H:2 * H].T]).astype(f16),
        "b1col": np.asarray(b1, np.float32).reshape(H, 1),
        "w2T": np.asarray(W2, np.float32).T.copy(),
        "b2col": np.asarray(b2, np.float32).reshape(OUT, 1),
    }

    x = np.asarray(raw, np.float32).reshape(N, t_len)
    feeds = []
    for c in range(NCORES):
        xs = x[c * nb:(c + 1) * nb]            # (nb, t)
        x0f = np.ones((2, t_len * nb), np.float32)
        x0f[0] = xs.T.reshape(-1)              # col t*nb+n
        x0r = np.ones((2, t_len * nb), np.float32)
        x0r[0] = xs.T[::-1].reshape(-1)        # col s*nb+n = x[n, t-1-s]
        feeds.append({"x0f": x0f.astype(f16), "x0r": x0r.astype(f16)})
    return shared, feeds


def kernel(raw, Wih0, Wih, Whh, bih, bhh, W1, b1, W2, b2):
    from concourse.bass_utils import run_bass_kernel_spmd

    if "prog" not in _CACHE:
        _CACHE["prog"] = _build_program()
    nc = _CACHE["prog"]

    shared, feeds = _prep_host(raw, Wih0, Wih, Whh, bih, bhh, W1, b1, W2, b2)
    in_maps = [dict(shared, **feeds[c]) for c in range(NCORES)]
    res = run_bass_kernel_spmd(nc, in_maps, list(range(NCORES)),
                               **_CACHE.get("run_kwargs", {}))
    _CACHE["last_results"] = res
    outs = [np.asarray(res.results[c]["out"], np.float32) for c in range(NCORES)]
    full = np.concatenate(outs, axis=1)        # (8, 400)
    return np.ascontiguousarray(full.T).reshape(B, KSEQ, OUT).astype(np.float32)
